# revision 36
# baseline (speedup 1.0000x reference)
"""Trainium2 Bass kernel for nn_ContextPromptGenerator.

Math restructure: the reference's cumsum + gathered-prefix pooling is linear,
so each pooled bin j of sample b is a masked segment-sum over tokens:

  pooled[b,j,:] = ( sum_{t in [max(start-1,0), end-1)} x[b,t,:] @ Wd
                    + cnt[b,j]*bd + ind[b,j]*(ctx_mean[b] @ Wc + bc) ) / S[b,j]

with start/end the adaptive-pool bin edges, cnt the number of hidden tokens in
the bin, ind whether the bin includes the prepended context token, S = end-start.
Contraction order is swapped: masked segment-sums of x against 0/1 masks come
FIRST, then the 4096->1024 down-projection runs on [64, 4096] per core instead
of [4096, 4096] (275 GFLOP -> ~18 GFLOP; kernel is HBM-read-bound).

Numerics: tolerance is 2e-2 so single-precision fp16 weights suffice (end-to-
end ~3e-4); the bin sums tolerate fp8: x and the gathered context embeddings
ship as float8 e3m4 (4-bit mantissa), which measures ~1.3e-2 end-to-end on the
fixed harness inputs while halving the dominant HBM stream. Embedding rows are
pre-scaled by 16 on the host (undone by a 1/16 mask) to keep them in e3m4's
normal range. All matmuls accumulate in fp32 PSUM.

Sharding: data-parallel, 2 samples per core; the pairing is chosen by a small
matching search that jointly minimizes the padded token count T and padded
context count Tc (both drive HBM bytes). The host packs only rows each sample
needs - pure data movement; every FLOP runs on device.
"""

import numpy as np
from contextlib import ExitStack

import ml_dtypes

import concourse.bass as bass
import concourse.mybir as mybir
import concourse.tile as tile
from concourse import bacc
from concourse.masks import make_identity
from concourse.bass_utils import run_bass_kernel_spmd

F32 = mybir.dt.float32
F16 = mybir.dt.float16
F8E3 = mybir.dt.float8e3
NP_E3M4 = ml_dtypes.float8_e3m4

B, S, C, H, D, V, P = 16, 2048, 512, 4096, 1024, 32000, 32
NC = 8          # cores
SPC = 2         # samples per core
M = SPC * P     # 64 output rows per core
HT = H // 128   # 32 h-tiles
DT = D // 128   # 8 d-tiles
ESCALE = 16.0   # embedding pre-scale so e3m4 stays in its normal range

_cache = {}


def _build(T, Tc):
    """Build the per-core SPMD Bass program.

    T  = packed hidden rows per core (multiple of 128)
    Tc = packed context-embedding rows per core (multiple of 128)
    """
    nc = bacc.Bacc(None, target_bir_lowering=False)

    xq_d = nc.dram_tensor("xq", [T, H], F8E3, kind="ExternalInput")
    mx_d = nc.dram_tensor("mx", [T, M], F16, kind="ExternalInput")
    eh_d = nc.dram_tensor("eh", [Tc, H], F8E3, kind="ExternalInput")
    cm_d = nc.dram_tensor("cm", [Tc, SPC], F16, kind="ExternalInput")
    wc_d = nc.dram_tensor("wc", [H, D], F8E3, kind="ExternalInput")
    wd_d = nc.dram_tensor("wd", [H, D], F16, kind="ExternalInput")
    wu_d = nc.dram_tensor("wu", [D, H], F16, kind="ExternalInput")
    bdc_d = nc.dram_tensor("bdc", [2, D], F16, kind="ExternalInput")
    bu_d = nc.dram_tensor("bur", [2, H // 2], F16, kind="ExternalInput")
    aug_d = nc.dram_tensor("aug", [4, M], F16, kind="ExternalInput")
    sinv_d = nc.dram_tensor("sinv", [M, 1], F32, kind="ExternalInput")
    out_d = nc.dram_tensor("out", [M, H], F16, kind="ExternalOutput")

    KT = T // 128    # x k-tiles
    KC = Tc // 128   # emb k-tiles

    HH = H // 2

    with tile.TileContext(nc) as tc, ExitStack() as ctx:
        const = ctx.enter_context(tc.tile_pool(name="const", bufs=1))
        # eh and Wc fully resident: once buffered, A/B matmuls run with no
        # per-tile waits, so the PE clock ramps and stays at full p-state
        ehpool = ctx.enter_context(tc.tile_pool(name="ehpool", bufs=KC))
        xpool = ctx.enter_context(tc.tile_pool(name="xpool", bufs=5))
        wcpool = ctx.enter_context(tc.tile_pool(name="wcpool", bufs=HT))
        wpool = ctx.enter_context(tc.tile_pool(name="wpool", bufs=22))
        wupool = ctx.enter_context(tc.tile_pool(name="wupool", bufs=8))
        mpool = ctx.enter_context(tc.tile_pool(name="mpool", bufs=8))
        keep = ctx.enter_context(tc.tile_pool(name="keep", bufs=1))

        # ---- phase A: ctx_sumT[h, s] = sum_r emb[r, h] * cm[r, s] ----
        # emb tiles stationary (e3m4), 1/16 mask moving; out 32 slices [128,2]
        with tc.tile_pool(name="psA", bufs=1, space="PSUM") as psA:
            ps_ctx = psA.tile([128, HT * SPC], F32)  # 1 bank
            for k in range(KC):
                eht = ehpool.tile([128, H], F8E3, tag="eh")
                nc.sync.dma_start(out=eht, in_=eh_d[128 * k:128 * (k + 1), :])
                cmt = mpool.tile([128, SPC], F16, tag="cm")
                nc.sync.dma_start(out=cmt, in_=cm_d[128 * k:128 * (k + 1), :])
                for hc in range(HT):
                    nc.tensor.matmul(
                        ps_ctx[:, SPC * hc:SPC * (hc + 1)],
                        eht[:, 128 * hc:128 * (hc + 1)],
                        cmt,
                        start=(k == 0 and hc == 0),
                        stop=(k == KC - 1),
                    )
            cs_h = keep.tile([128, HT * SPC], F16)
            nc.vector.tensor_copy(cs_h, ps_ctx)

        # ---- prefetch Wd on the Activation engine's HW DMA queue, gated on
        # phase A's output so it doesn't steal early bandwidth from eh/Wc.
        # The SP queue carries eh/Wc/x/Wu in program order; the second queue
        # keeps HBM busy whenever one queue head-blocks on a recycled buffer.
        gate = keep.tile([1, 2], F16)
        nc.scalar.activation(gate, cs_h[0:1, 0:2],
                             mybir.ActivationFunctionType.Silu)
        wd_tiles = []
        for k in range(HT):
            wdt = wpool.tile([128, D], F16, tag="w", name=f"wd{k}")
            nc.scalar.dma_start(out=wdt, in_=wd_d[128 * k:128 * (k + 1), :])
            wd_tiles.append(wdt)

        # small inputs + constants, issued after the first big DMAs so the
        # data queue starts on the critical stream immediately
        ident = const.tile([128, 128], F16)
        make_identity(nc, ident)
        ones1 = const.tile([1, M], F16)
        nc.vector.memset(ones1, 1.0)
        aug_sb = keep.tile([4, M], F16)
        nc.sync.dma_start(out=aug_sb, in_=aug_d[:, :])
        sinv_sb = keep.tile([M, 1], F32)
        nc.sync.dma_start(out=sinv_sb, in_=sinv_d[:, :])
        bu_sb = []
        for h in range(2):
            but = keep.tile([1, H // 2], F16, tag=f"bu{h}", name=f"bu{h}")
            nc.sync.dma_start(out=but, in_=bu_d[h:h + 1, :])
            bu_sb.append(but)
        # augmented-rhs rows: 0=ctxWcSum[a]*16, 1=ctxWcSum[b]*16, 2=bd, 3=bc
        augr_sb = keep.tile([4, D], F16)
        nc.sync.dma_start(out=augr_sb[2:4, :], in_=bdc_d[:, :])

        # ---- phase B: ctxWcSum rows [2, 1024] = cs.T @ Wc ----
        with tc.tile_pool(name="psB", bufs=1, space="PSUM") as psB:
            ps_cd = psB.tile([SPC, D], F32)  # 2 banks
            for k in range(HT):
                wct = wcpool.tile([128, D], F8E3, tag="wc")
                nc.sync.dma_start(out=wct, in_=wc_d[128 * k:128 * (k + 1), :])
                csk = cs_h[:, SPC * k:SPC * (k + 1)]
                for nb in range(2):
                    nc.tensor.matmul(
                        ps_cd[:, 512 * nb:512 * (nb + 1)],
                        csk, wct[:, 512 * nb:512 * (nb + 1)],
                        start=(k == 0),
                        stop=(k == HT - 1),
                    )
            nc.vector.tensor_copy(augr_sb[0:2, :], ps_cd)

        # ---- phase X: xsumT[h, j] = sum_t x[t, h] * mx01[t, j] ----
        # x tiles stationary (e3m4), 0/1 mask moving; out 32 slices [128,64]
        with tc.tile_pool(name="psX", bufs=1, space="PSUM") as psX:
            ps_xs = psX.tile([128, HT * M], F32)  # 4 banks, 8 slices per bank
            for k in range(KT):
                xt = xpool.tile([128, H], F8E3, tag="x")
                nc.sync.dma_start(out=xt, in_=xq_d[128 * k:128 * (k + 1), :])
                mxt = mpool.tile([128, M], F16, tag="mx")
                nc.sync.dma_start(out=mxt, in_=mx_d[128 * k:128 * (k + 1), :])
                for hc in range(HT):
                    nc.tensor.matmul(
                        ps_xs[:, M * hc:M * (hc + 1)],
                        xt[:, 128 * hc:128 * (hc + 1)],
                        mxt,
                        start=(k == 0 and hc % 8 == 0),
                        stop=(k == KT - 1),
                    )
            xs_h = keep.tile([128, HT * M], F16)
            for q in range(4):
                nc.vector.tensor_copy(
                    xs_h[:, 512 * q:512 * (q + 1)],
                    ps_xs[:, 512 * q:512 * (q + 1)])

        # ---- phase D: pooled[j, d] = xsum.T @ Wd + aug; silu ----
        # Wu rides the SP queue behind x: its first tiles land while D/E run,
        # the rest stream just-in-time as U consumes them (out DMAs are the
        # only thing behind them, and those wait on U anyway)
        wu_tiles = []
        for half in range(2):
            for dc in range(DT):
                wut = wupool.tile([128, HH], F16, tag="wu",
                                  name=f"wu{half}_{dc}")
                nc.sync.dma_start(
                    out=wut,
                    in_=wu_d[128 * dc:128 * (dc + 1),
                             HH * half:HH * (half + 1)])
                wu_tiles.append(wut)

        silu_h = keep.tile([M, D], F16)
        with tc.tile_pool(name="psD", bufs=1, space="PSUM") as psD:
            ps_pool = psD.tile([M, D], F32)  # 2 banks
            for k in range(HT):
                wdt = wd_tiles[k]
                xsk = xs_h[:, M * k:M * (k + 1)]
                for nb in range(2):
                    nc.tensor.matmul(
                        ps_pool[:, 512 * nb:512 * (nb + 1)],
                        xsk, wdt[:, 512 * nb:512 * (nb + 1)],
                        start=(k == 0),
                        stop=False,
                    )
            # aug last: it only needs phase B's augr by the end of the group
            for nb in range(2):
                nc.tensor.matmul(
                    ps_pool[:, 512 * nb:512 * (nb + 1)],
                    aug_sb,
                    augr_sb[:, 512 * nb:512 * (nb + 1)],
                    start=False, stop=True,
                )
            # scale by 1/S and silu in one ACT op per bank
            for nb in range(2):
                nc.scalar.activation(
                    silu_h[:, 512 * nb:512 * (nb + 1)],
                    ps_pool[:, 512 * nb:512 * (nb + 1)],
                    mybir.ActivationFunctionType.Silu,
                    scale=sinv_sb,
                )

        # ---- phase E: siluT slices [128, 64] per d-tile ----
        sT_h = keep.tile([128, DT * M], F16)
        with tc.tile_pool(name="psE", bufs=2, space="PSUM") as psE:
            for dc in range(DT):
                pst = psE.tile([128, M], F16, tag="tr")
                nc.tensor.transpose(
                    pst, silu_h[:, 128 * dc:128 * (dc + 1)],
                    ident[0:M, 0:M])
                nc.vector.tensor_copy(sT_h[:, M * dc:M * (dc + 1)], pst)

        # ---- phase U: out[j, h] = siluT.T @ Wu + bu ----
        out_sb = keep.tile([M, H], F16)
        with tc.tile_pool(name="psU", bufs=2, space="PSUM") as psU:
            for half in range(2):
                ps_out = psU.tile([M, HH], F32, tag="o")  # 4 banks
                # bias first so the last Wu tile closes the group
                for nb in range(HH // 512):
                    nc.tensor.matmul(
                        ps_out[:, 512 * nb:512 * (nb + 1)],
                        ones1,
                        bu_sb[half][:, 512 * nb:512 * (nb + 1)],
                        start=True, stop=False,
                    )
                for dc in range(DT):
                    wut = wu_tiles[half * DT + dc]
                    sk = sT_h[:, M * dc:M * (dc + 1)]
                    for nb in range(HH // 512):
                        nc.tensor.matmul(
                            ps_out[:, 512 * nb:512 * (nb + 1)],
                            sk, wut[:, 512 * nb:512 * (nb + 1)],
                            start=False,
                            stop=(dc == DT - 1),
                        )
                for nb in range(HH // 512):
                    sl = slice(HH * half + 512 * nb, HH * half + 512 * (nb + 1))
                    nc.vector.tensor_copy(out_sb[:, sl], ps_out[:, 512 * nb:512 * (nb + 1)])
                    nc.sync.dma_start(out=out_d[:, sl], in_=out_sb[:, sl])

    nc.finalize()
    return nc


def _roundup(v, m):
    return max(m, ((int(v) + m - 1) // m) * m)


def _find_pairs(seq, clen):
    """Perfect matching of the 16 samples into 8 core-pairs minimizing the
    padded HBM bytes T*(row bytes of x+mask) + Tc*(row bytes of emb+cmask)."""
    n = len(seq)
    order = sorted(range(n), key=lambda i: -int(seq[i]))

    def match(Tt, Ct):
        used = [False] * n
        pairs = []

        def bt(idx):
            while idx < n and used[order[idx]]:
                idx += 1
            if idx == n:
                return True
            a = order[idx]
            used[a] = True
            for j in range(idx + 1, n):
                b = order[j]
                if used[b]:
                    continue
                if (seq[a] + seq[b] <= Tt and clen[a] + clen[b] <= Ct):
                    used[b] = True
                    pairs.append((a, b))
                    if bt(idx + 1):
                        return True
                    used[b] = False
                    pairs.pop()
            used[a] = False
            return False

        return list(pairs) if bt(0) else None

    xrow = H + M * 2      # e3m4 x row + f16 mask row
    erow = H + SPC * 2    # e3m4 emb row + f16 cmask row
    t_lo = _roundup(max(int(v) for v in seq) + 1, 128)
    c_lo = _roundup(max(int(v) for v in clen) + 1, 128)
    best = None
    for Tt in range(t_lo, t_lo + 1024 + 1, 128):
        for Ct in range(c_lo, c_lo + 1024 + 1, 128):
            cost = Tt * xrow + Ct * erow
            if best is not None and cost >= best[0]:
                continue
            got = match(Tt, Ct)
            if got is not None:
                best = (cost, Tt, Ct, got)
    assert best is not None
    _, T, Tc, pairs = best
    return T, Tc, pairs


def kernel(**inputs):
    ids = np.asarray(inputs["context_ids"]).astype(np.int64)
    x = np.asarray(inputs["hidden_states"], dtype=np.float32)
    seq = np.asarray(inputs["seq_lengths"]).astype(np.int64)
    clen = np.asarray(inputs["context_lengths"]).astype(np.int64)
    emb = np.asarray(inputs["embed_table"], dtype=np.float32)
    Wc = np.ascontiguousarray(inputs["Wc"], dtype=np.float32)
    bc = np.asarray(inputs["bc"], dtype=np.float32)
    Wd = np.ascontiguousarray(inputs["Wd"], dtype=np.float32)
    bd = np.asarray(inputs["bd"], dtype=np.float32)
    Wu = np.ascontiguousarray(inputs["Wu"], dtype=np.float32)
    bu = np.asarray(inputs["bu"], dtype=np.float32)

    assert x.shape == (B, S, H) and ids.shape == (B, C)

    # per-sample bin geometry
    L = seq + 1
    jj = np.arange(P, dtype=np.int64)
    start = (jj[None, :] * L[:, None]) // P            # [B,P]
    end = ((jj[None, :] + 1) * L[:, None] + P - 1) // P
    Sj = (end - start).astype(np.float32)
    lo = np.maximum(start - 1, 0)
    hi = end - 1
    cnt = (hi - lo).astype(np.float32)
    ind = (start == 0).astype(np.float32)

    T, Tc, pairs = _find_pairs(seq, clen)

    key = (T, Tc)
    if key not in _cache:
        _cache[key] = _build(T, Tc)
    nc = _cache[key]

    wc8 = (Wc * ESCALE).astype(NP_E3M4)
    wd16 = Wd.astype(np.float16)
    wu16 = Wu.astype(np.float16)
    bdc = np.stack([bd, bc]).astype(np.float16)
    bu_r = bu.reshape(2, H // 2).astype(np.float16)

    in_maps = []
    for a, b in pairs:
        sa, sb = int(seq[a]), int(seq[b])
        ca, cb = max(1, int(clen[a])), max(1, int(clen[b]))
        xp = np.zeros((T, H), NP_E3M4)
        xp[:sa] = x[a, :sa].astype(NP_E3M4)
        xp[sa:sa + sb] = x[b, :sb].astype(NP_E3M4)
        t = np.arange(T, dtype=np.int64)[:, None]
        mx = np.zeros((T, M), np.float16)
        mx[:, :P] = ((t >= lo[a][None, :]) & (t < hi[a][None, :]))
        mx[:, P:] = ((t - sa >= lo[b][None, :]) & (t - sa < hi[b][None, :])
                     & (t >= sa))
        ep = np.zeros((Tc, H), NP_E3M4)
        ep[:ca] = (emb[ids[a, :ca]] * ESCALE).astype(NP_E3M4)
        ep[ca:ca + cb] = (emb[ids[b, :cb]] * ESCALE).astype(NP_E3M4)
        cm = np.zeros((Tc, SPC), np.float16)
        cm[:ca, 0] = 1.0 / ESCALE
        cm[ca:ca + cb, 1] = 1.0 / ESCALE
        # rows 0/1 also undo the ESCALE folded into the fp8 Wc
        aug = np.zeros((4, M), np.float16)
        aug[0, :P] = ind[a] / ca / ESCALE
        aug[1, P:] = ind[b] / cb / ESCALE
        aug[2, :P] = cnt[a]
        aug[2, P:] = cnt[b]
        aug[3, :P] = ind[a]
        aug[3, P:] = ind[b]
        sinv = np.concatenate([1.0 / Sj[a], 1.0 / Sj[b]]).reshape(M, 1)
        in_maps.append({
            "xq": xp, "mx": mx, "eh": ep, "cm": cm,
            "wc": wc8, "wd": wd16, "wu": wu16,
            "bdc": bdc, "bur": bu_r,
            "aug": aug, "sinv": sinv.astype(np.float32),
        })

    res = run_bass_kernel_spmd(nc, in_maps, core_ids=list(range(NC)))
    _cache["last_result"] = res

    out = np.empty((B, P, H), np.float32)
    for c, (a, b) in enumerate(pairs):
        o = np.asarray(res.results[c]["out"]).astype(np.float32)
        out[a] = o[:P]
        out[b] = o[P:]
    return out


# revision 39
# speedup vs baseline: 1.0754x; 1.0754x over previous
"""Trainium2 Bass kernel for nn_ContextPromptGenerator.

Math restructure: the reference's cumsum + gathered-prefix pooling is linear,
so each pooled bin j of sample b is a masked segment-sum over tokens:

  pooled[b,j,:] = ( sum_{t in [max(start-1,0), end-1)} x[b,t,:] @ Wd
                    + cnt[b,j]*bd + ind[b,j]*(ctx_mean[b] @ Wc + bc) ) / S[b,j]

with start/end the adaptive-pool bin edges, cnt the number of hidden tokens in
the bin, ind whether the bin includes the prepended context token, S = end-start.
Contraction order is swapped: masked segment-sums of x against 0/1 masks come
FIRST, then the 4096->1024 down-projection runs on [64, 4096] per core instead
of [4096, 4096] (275 GFLOP -> ~18 GFLOP; kernel is HBM-read-bound).

Numerics: tolerance is 2e-2 so single-precision fp16 weights suffice (end-to-
end ~3e-4); the bin sums tolerate fp8: x and the gathered context embeddings
ship as float8 e3m4 (4-bit mantissa), which measures ~1.3e-2 end-to-end on the
fixed harness inputs while halving the dominant HBM stream. Embedding rows are
pre-scaled by 16 on the host (undone by a 1/16 mask) to keep them in e3m4's
normal range. All matmuls accumulate in fp32 PSUM.

Sharding: data-parallel, 2 samples per core; the pairing is chosen by a small
matching search that jointly minimizes the padded token count T and padded
context count Tc (both drive HBM bytes). The host packs only rows each sample
needs - pure data movement; every FLOP runs on device.
"""

import numpy as np
from contextlib import ExitStack

import ml_dtypes

import concourse.bass as bass
import concourse.mybir as mybir
import concourse.tile as tile
from concourse import bacc
from concourse.masks import make_identity
from concourse.bass_utils import run_bass_kernel_spmd

F32 = mybir.dt.float32
F16 = mybir.dt.float16
F8E3 = mybir.dt.float8e3
NP_E3M4 = ml_dtypes.float8_e3m4

B, S, C, H, D, V, P = 16, 2048, 512, 4096, 1024, 32000, 32
NC = 8          # cores
SPC = 2         # samples per core
M = SPC * P     # 64 output rows per core
HT = H // 128   # 32 h-tiles
DT = D // 128   # 8 d-tiles
ESCALE = 16.0   # embedding pre-scale so e3m4 stays in its normal range

_cache = {}


def _build(T, Tc):
    """Build the per-core SPMD Bass program.

    T  = packed hidden rows per core (multiple of 128)
    Tc = packed context-embedding rows per core (multiple of 128)
    """
    nc = bacc.Bacc(None, target_bir_lowering=False)

    xq_d = nc.dram_tensor("xq", [T, H], F8E3, kind="ExternalInput")
    mx_d = nc.dram_tensor("mx", [T, M], F16, kind="ExternalInput")
    eh_d = nc.dram_tensor("eh", [Tc, H], F8E3, kind="ExternalInput")
    cm_d = nc.dram_tensor("cm", [Tc, SPC], F16, kind="ExternalInput")
    wc_d = nc.dram_tensor("wc", [H, D], F8E3, kind="ExternalInput")
    wd_d = nc.dram_tensor("wd", [H, D], F16, kind="ExternalInput")
    wu_d = nc.dram_tensor("wu", [D, H], F16, kind="ExternalInput")
    bdc_d = nc.dram_tensor("bdc", [2, D], F16, kind="ExternalInput")
    bu_d = nc.dram_tensor("bur", [2, H // 2], F16, kind="ExternalInput")
    aug_d = nc.dram_tensor("aug", [4, M], F16, kind="ExternalInput")
    sinv_d = nc.dram_tensor("sinv", [M, 1], F32, kind="ExternalInput")
    out_d = nc.dram_tensor("out", [M, H], F16, kind="ExternalOutput")

    KT = T // 128    # x k-tiles
    KC = Tc // 128   # emb k-tiles

    HH = H // 2

    with tile.TileContext(nc) as tc, ExitStack() as ctx:
        const = ctx.enter_context(tc.tile_pool(name="const", bufs=1))
        # Single DMA queue ordered by need-time. eh and Wc fully resident so
        # A/B matmuls never wait per-tile (the PE clock ramps and stays up);
        # x/Wd/Wu trickle just-in-time behind them in ring order.
        ehpool = ctx.enter_context(tc.tile_pool(name="ehpool", bufs=KC))
        xpool = ctx.enter_context(tc.tile_pool(name="xpool", bufs=10))
        wcpool = ctx.enter_context(tc.tile_pool(name="wcpool", bufs=HT))
        wpool = ctx.enter_context(tc.tile_pool(name="wpool", bufs=12))
        wupool = ctx.enter_context(tc.tile_pool(name="wupool", bufs=8))
        mpool = ctx.enter_context(tc.tile_pool(name="mpool", bufs=8))
        keep = ctx.enter_context(tc.tile_pool(name="keep", bufs=1))

        # ---- phase A: ctx_sumT[h, s] = sum_r emb[r, h] * cm[r, s] ----
        # emb tiles stationary (e3m4), 1/16 mask moving; out 32 slices [128,2]
        with tc.tile_pool(name="psA", bufs=1, space="PSUM") as psA:
            ps_ctx = psA.tile([128, HT * SPC], F32)  # 1 bank
            for k in range(KC):
                eht = ehpool.tile([128, H], F8E3, tag="eh")
                nc.sync.dma_start(out=eht, in_=eh_d[128 * k:128 * (k + 1), :])
                cmt = mpool.tile([128, SPC], F16, tag="cm")
                nc.sync.dma_start(out=cmt, in_=cm_d[128 * k:128 * (k + 1), :])
                for hc in range(HT):
                    nc.tensor.matmul(
                        ps_ctx[:, SPC * hc:SPC * (hc + 1)],
                        eht[:, 128 * hc:128 * (hc + 1)],
                        cmt,
                        start=(k == 0 and hc == 0),
                        stop=(k == KC - 1),
                    )
            cs_h = keep.tile([128, HT * SPC], F16)
            nc.vector.tensor_copy(cs_h, ps_ctx)

        # small inputs + constants, issued after the first big DMAs so the
        # data queue starts on the critical stream immediately
        ident = const.tile([128, 128], F16)
        make_identity(nc, ident)
        ones1 = const.tile([1, M], F16)
        nc.vector.memset(ones1, 1.0)
        aug_sb = keep.tile([4, M], F16)
        nc.sync.dma_start(out=aug_sb, in_=aug_d[:, :])
        sinv_sb = keep.tile([M, 1], F32)
        nc.sync.dma_start(out=sinv_sb, in_=sinv_d[:, :])
        bu_sb = []
        for h in range(2):
            but = keep.tile([1, H // 2], F16, tag=f"bu{h}", name=f"bu{h}")
            nc.sync.dma_start(out=but, in_=bu_d[h:h + 1, :])
            bu_sb.append(but)
        # augmented-rhs rows: 0=ctxWcSum[a]*16, 1=ctxWcSum[b]*16, 2=bd, 3=bc
        augr_sb = keep.tile([4, D], F16)
        nc.sync.dma_start(out=augr_sb[2:4, :], in_=bdc_d[:, :])

        # ---- phase B: ctxWcSum rows [2, 1024] = cs.T @ Wc ----
        with tc.tile_pool(name="psB", bufs=1, space="PSUM") as psB:
            ps_cd = psB.tile([SPC, D], F32)  # 2 banks
            for k in range(HT):
                wct = wcpool.tile([128, D], F8E3, tag="wc")
                nc.sync.dma_start(out=wct, in_=wc_d[128 * k:128 * (k + 1), :])
                csk = cs_h[:, SPC * k:SPC * (k + 1)]
                for nb in range(2):
                    nc.tensor.matmul(
                        ps_cd[:, 512 * nb:512 * (nb + 1)],
                        csk, wct[:, 512 * nb:512 * (nb + 1)],
                        start=(k == 0),
                        stop=(k == HT - 1),
                    )
            nc.vector.tensor_copy(augr_sb[0:2, :], ps_cd)

        # ---- phase X: xsumT[h, j] = sum_t x[t, h] * mx01[t, j] ----
        # x tiles stationary (e3m4), 0/1 mask moving; out 32 slices [128,64]
        with tc.tile_pool(name="psX", bufs=1, space="PSUM") as psX:
            ps_xs = psX.tile([128, HT * M], F32)  # 4 banks, 8 slices per bank
            for k in range(KT):
                xt = xpool.tile([128, H], F8E3, tag="x")
                nc.sync.dma_start(out=xt, in_=xq_d[128 * k:128 * (k + 1), :])
                mxt = mpool.tile([128, M], F16, tag="mx")
                nc.sync.dma_start(out=mxt, in_=mx_d[128 * k:128 * (k + 1), :])
                for hc in range(HT):
                    nc.tensor.matmul(
                        ps_xs[:, M * hc:M * (hc + 1)],
                        xt[:, 128 * hc:128 * (hc + 1)],
                        mxt,
                        start=(k == 0 and hc % 8 == 0),
                        stop=(k == KT - 1),
                    )
            xs_h = keep.tile([128, HT * M], F16)
            for q in range(4):
                nc.vector.tensor_copy(
                    xs_h[:, 512 * q:512 * (q + 1)],
                    ps_xs[:, 512 * q:512 * (q + 1)])

        # ---- phase D: pooled[j, d] = xsum.T @ Wd + aug; silu ----
        # Wd then Wu behind x in ring order: they trickle in during X/D/E,
        # consumed just-in-time (out DMAs are the only thing behind them,
        # and those wait on U anyway)
        wd_tiles = []
        for k in range(HT):
            wdt = wpool.tile([128, D], F16, tag="w", name=f"wd{k}")
            nc.sync.dma_start(out=wdt, in_=wd_d[128 * k:128 * (k + 1), :])
            wd_tiles.append(wdt)
        wu_tiles = []
        for half in range(2):
            for dc in range(DT):
                wut = wupool.tile([128, HH], F16, tag="wu",
                                  name=f"wu{half}_{dc}")
                nc.sync.dma_start(
                    out=wut,
                    in_=wu_d[128 * dc:128 * (dc + 1),
                             HH * half:HH * (half + 1)])
                wu_tiles.append(wut)

        silu_h = keep.tile([M, D], F16)
        with tc.tile_pool(name="psD", bufs=1, space="PSUM") as psD:
            ps_pool = psD.tile([M, D], F32)  # 2 banks
            for k in range(HT):
                wdt = wd_tiles[k]
                xsk = xs_h[:, M * k:M * (k + 1)]
                for nb in range(2):
                    nc.tensor.matmul(
                        ps_pool[:, 512 * nb:512 * (nb + 1)],
                        xsk, wdt[:, 512 * nb:512 * (nb + 1)],
                        start=(k == 0),
                        stop=False,
                    )
            # aug last: it only needs phase B's augr by the end of the group
            for nb in range(2):
                nc.tensor.matmul(
                    ps_pool[:, 512 * nb:512 * (nb + 1)],
                    aug_sb,
                    augr_sb[:, 512 * nb:512 * (nb + 1)],
                    start=False, stop=True,
                )
            # scale by 1/S and silu in one ACT op per bank
            for nb in range(2):
                nc.scalar.activation(
                    silu_h[:, 512 * nb:512 * (nb + 1)],
                    ps_pool[:, 512 * nb:512 * (nb + 1)],
                    mybir.ActivationFunctionType.Silu,
                    scale=sinv_sb,
                )

        # ---- phase E: siluT slices [128, 64] per d-tile ----
        sT_h = keep.tile([128, DT * M], F16)
        with tc.tile_pool(name="psE", bufs=2, space="PSUM") as psE:
            for dc in range(DT):
                pst = psE.tile([128, M], F16, tag="tr")
                nc.tensor.transpose(
                    pst, silu_h[:, 128 * dc:128 * (dc + 1)],
                    ident[0:M, 0:M])
                nc.vector.tensor_copy(sT_h[:, M * dc:M * (dc + 1)], pst)

        # ---- phase U: out[j, h] = siluT.T @ Wu + bu ----
        out_sb = keep.tile([M, H], F16)
        with tc.tile_pool(name="psU", bufs=2, space="PSUM") as psU:
            for half in range(2):
                ps_out = psU.tile([M, HH], F32, tag="o")  # 4 banks
                # bias first so the last Wu tile closes the group
                for nb in range(HH // 512):
                    nc.tensor.matmul(
                        ps_out[:, 512 * nb:512 * (nb + 1)],
                        ones1,
                        bu_sb[half][:, 512 * nb:512 * (nb + 1)],
                        start=True, stop=False,
                    )
                for dc in range(DT):
                    wut = wu_tiles[half * DT + dc]
                    sk = sT_h[:, M * dc:M * (dc + 1)]
                    for nb in range(HH // 512):
                        nc.tensor.matmul(
                            ps_out[:, 512 * nb:512 * (nb + 1)],
                            sk, wut[:, 512 * nb:512 * (nb + 1)],
                            start=False,
                            stop=(dc == DT - 1),
                        )
                for nb in range(HH // 512):
                    sl = slice(HH * half + 512 * nb, HH * half + 512 * (nb + 1))
                    nc.vector.tensor_copy(out_sb[:, sl], ps_out[:, 512 * nb:512 * (nb + 1)])
                    nc.sync.dma_start(out=out_d[:, sl], in_=out_sb[:, sl])

    nc.finalize()
    return nc


def _roundup(v, m):
    return max(m, ((int(v) + m - 1) // m) * m)


def _find_pairs(seq, clen):
    """Perfect matching of the 16 samples into 8 core-pairs minimizing the
    padded HBM bytes T*(row bytes of x+mask) + Tc*(row bytes of emb+cmask)."""
    n = len(seq)
    order = sorted(range(n), key=lambda i: -int(seq[i]))

    def match(Tt, Ct):
        used = [False] * n
        pairs = []

        def bt(idx):
            while idx < n and used[order[idx]]:
                idx += 1
            if idx == n:
                return True
            a = order[idx]
            used[a] = True
            for j in range(idx + 1, n):
                b = order[j]
                if used[b]:
                    continue
                if (seq[a] + seq[b] <= Tt and clen[a] + clen[b] <= Ct):
                    used[b] = True
                    pairs.append((a, b))
                    if bt(idx + 1):
                        return True
                    used[b] = False
                    pairs.pop()
            used[a] = False
            return False

        return list(pairs) if bt(0) else None

    xrow = H + M * 2      # e3m4 x row + f16 mask row
    erow = H + SPC * 2    # e3m4 emb row + f16 cmask row
    t_lo = _roundup(max(int(v) for v in seq) + 1, 128)
    c_lo = _roundup(max(int(v) for v in clen) + 1, 128)
    best = None
    for Tt in range(t_lo, t_lo + 1024 + 1, 128):
        for Ct in range(c_lo, c_lo + 1024 + 1, 128):
            cost = Tt * xrow + Ct * erow
            if best is not None and cost >= best[0]:
                continue
            got = match(Tt, Ct)
            if got is not None:
                best = (cost, Tt, Ct, got)
    assert best is not None
    _, T, Tc, pairs = best
    return T, Tc, pairs


def kernel(**inputs):
    ids = np.asarray(inputs["context_ids"]).astype(np.int64)
    x = np.asarray(inputs["hidden_states"], dtype=np.float32)
    seq = np.asarray(inputs["seq_lengths"]).astype(np.int64)
    clen = np.asarray(inputs["context_lengths"]).astype(np.int64)
    emb = np.asarray(inputs["embed_table"], dtype=np.float32)
    Wc = np.ascontiguousarray(inputs["Wc"], dtype=np.float32)
    bc = np.asarray(inputs["bc"], dtype=np.float32)
    Wd = np.ascontiguousarray(inputs["Wd"], dtype=np.float32)
    bd = np.asarray(inputs["bd"], dtype=np.float32)
    Wu = np.ascontiguousarray(inputs["Wu"], dtype=np.float32)
    bu = np.asarray(inputs["bu"], dtype=np.float32)

    assert x.shape == (B, S, H) and ids.shape == (B, C)

    # per-sample bin geometry
    L = seq + 1
    jj = np.arange(P, dtype=np.int64)
    start = (jj[None, :] * L[:, None]) // P            # [B,P]
    end = ((jj[None, :] + 1) * L[:, None] + P - 1) // P
    Sj = (end - start).astype(np.float32)
    lo = np.maximum(start - 1, 0)
    hi = end - 1
    cnt = (hi - lo).astype(np.float32)
    ind = (start == 0).astype(np.float32)

    T, Tc, pairs = _find_pairs(seq, clen)

    key = (T, Tc)
    if key not in _cache:
        _cache[key] = _build(T, Tc)
    nc = _cache[key]

    wc8 = (Wc * ESCALE).astype(NP_E3M4)
    wd16 = Wd.astype(np.float16)
    wu16 = Wu.astype(np.float16)
    bdc = np.stack([bd, bc]).astype(np.float16)
    bu_r = bu.reshape(2, H // 2).astype(np.float16)

    in_maps = []
    for a, b in pairs:
        sa, sb = int(seq[a]), int(seq[b])
        ca, cb = max(1, int(clen[a])), max(1, int(clen[b]))
        xp = np.zeros((T, H), NP_E3M4)
        xp[:sa] = x[a, :sa].astype(NP_E3M4)
        xp[sa:sa + sb] = x[b, :sb].astype(NP_E3M4)
        t = np.arange(T, dtype=np.int64)[:, None]
        mx = np.zeros((T, M), np.float16)
        mx[:, :P] = ((t >= lo[a][None, :]) & (t < hi[a][None, :]))
        mx[:, P:] = ((t - sa >= lo[b][None, :]) & (t - sa < hi[b][None, :])
                     & (t >= sa))
        ep = np.zeros((Tc, H), NP_E3M4)
        ep[:ca] = (emb[ids[a, :ca]] * ESCALE).astype(NP_E3M4)
        ep[ca:ca + cb] = (emb[ids[b, :cb]] * ESCALE).astype(NP_E3M4)
        cm = np.zeros((Tc, SPC), np.float16)
        cm[:ca, 0] = 1.0 / ESCALE
        cm[ca:ca + cb, 1] = 1.0 / ESCALE
        # rows 0/1 also undo the ESCALE folded into the fp8 Wc
        aug = np.zeros((4, M), np.float16)
        aug[0, :P] = ind[a] / ca / ESCALE
        aug[1, P:] = ind[b] / cb / ESCALE
        aug[2, :P] = cnt[a]
        aug[2, P:] = cnt[b]
        aug[3, :P] = ind[a]
        aug[3, P:] = ind[b]
        sinv = np.concatenate([1.0 / Sj[a], 1.0 / Sj[b]]).reshape(M, 1)
        in_maps.append({
            "xq": xp, "mx": mx, "eh": ep, "cm": cm,
            "wc": wc8, "wd": wd16, "wu": wu16,
            "bdc": bdc, "bur": bu_r,
            "aug": aug, "sinv": sinv.astype(np.float32),
        })

    res = run_bass_kernel_spmd(nc, in_maps, core_ids=list(range(NC)))
    _cache["last_result"] = res

    out = np.empty((B, P, H), np.float32)
    for c, (a, b) in enumerate(pairs):
        o = np.asarray(res.results[c]["out"]).astype(np.float32)
        out[a] = o[:P]
        out[b] = o[P:]
    return out
